# revision 16
# baseline (speedup 1.0000x reference)
"""BVH router adapter kernel for Trainium2 (8 NeuronCores, token-parallel).

Computes, for hidden_states [16384, 2048]:
  x = RMSNorm(h) * norm_weight * scale * root
  bvh_logits  = x @ bvh_w.T  + bvh_b     (candidate selection, top-32)
  full_logits = x @ proj_w.T + proj_b
  full_probs  = softmax(full_logits)
  top-2 of full_probs restricted to the 32 bvh candidates
  weights = normalized top-2 * per_expert_scale[idx]

Device strategy (per core, 2048 tokens = 16 subtiles of 128):
  - norm folded into weights: Wcat = (norm_weight * W) * scale * root,
    per-token factor s = rsqrt(mean(h^2)+eps) applied as the exp() scale
    (selection is invariant to the positive per-token scale when biases
    are zero, so top-k runs on raw h @ Wcat logits).
  - x transposed on-chip via PE transpose (PSUM) + ACT copy to SBUF.
  - matmul schemes: "fp16x2" (x1@w1 + x1@w2, fp16, full-rate),
    "f32r" (single pass, TF32-ish), "f32" (exact, quarter-rate).
  - top-32 mask via 4 rounds of DVE max8 + match_replace; top-2 via
    max8 + max_index on candidate-masked logits.
"""

import numpy as np
import ml_dtypes

import concourse.bass as bass
import concourse.tile as tile
from concourse import mybir
from concourse.bass_utils import run_bass_kernel_spmd
from concourse.masks import make_identity

F32 = mybir.dt.float32
AOT = mybir.ActivationFunctionType
ALU = mybir.AluOpType

N_TOKENS = 16384
HIDDEN = 2048
E = 128          # experts
NCAND = 32
TOPK = 2
EPS = 1e-6
N_CORES = 8
TPC = N_TOKENS // N_CORES        # tokens per core = 2048
NSUB = TPC // 128                # subtiles per core = 16
NCHUNK = HIDDEN // 128           # contraction chunks = 16

SCHEME = "f32"                   # "f32" (exact) | "fp16x2" | "f32r"
NEG_BIG = -1e30

# how the per-subtile sum-of-squares pass is split across engines
SS_ACT = 1088                    # h[0:SS_ACT] on ACT, rest on DVE


def split_multi_waits(nc):
    """The walrus build in this environment supports only one sync-wait per
    instruction; hoist extra waits onto same-engine NOPs placed before it."""
    for func in nc.m.functions:
        for block in func.blocks:
            new_insts = []
            for inst in block.instructions:
                si = inst.sync_info
                if si is not None and len(si.on_wait) > 1:
                    waits = list(si.on_wait)
                    for j, w in enumerate(waits[:-1]):
                        nop = mybir.InstNoOp(
                            name=f"{inst.name}-ws{j}", ins=[], outs=[])
                        nop.engine = inst.engine
                        nop.sync_info = mybir.SyncInfo(on_wait=[w], on_update=[])
                        new_insts.append(nop)
                    inst.sync_info = mybir.SyncInfo(
                        on_wait=[waits[-1]], on_update=list(si.on_update))
                new_insts.append(inst)
            block.instructions = new_insts


def _mm_dtypes(scheme):
    if scheme == "fp16x2":
        return [mybir.dt.float16, mybir.dt.float16]
    if scheme == "f32r":
        return [mybir.dt.float32r]
    return [mybir.dt.float32]


def build_program(scheme=SCHEME, repeat=1, ablate=(), bufs=None):
    """One SPMD program; every core runs it on its own token shard.

    repeat > 1 wraps the whole per-core workload in a hardware loop that
    recomputes identical results; used only for wall-clock timing."""
    import contextlib
    B = {"xin": 3, "xt": 2, "pst": 3, "psl": 2, "ssc": 2, "ex": 2, "sm": 4,
         "outp": 3, "pst_w": 1024}
    if bufs:
        B.update(bufs)
    mm_dts = _mm_dtypes(scheme)
    n_pass = len(mm_dts)
    xt_dt = mm_dts[0]

    # rsqrt post-scale folded into the Sqrt activation:
    #   s = 1 / (WPRE * sqrt(ss/HIDDEN + eps))
    #     = 1 / sqrt(ss * (WPRE^2/HIDDEN) + WPRE^2*eps)
    wpre = 256.0 if scheme == "fp16x2" else 1.0
    sq_scale = float(wpre * wpre / HIDDEN)
    sq_bias = float(wpre * wpre * EPS)

    nc = bass.Bass("TRN2", target_bir_lowering=False, debug=False)
    x_d = nc.dram_tensor("x", [TPC, HIDDEN], F32, kind="ExternalInput")
    w_d = [nc.dram_tensor(f"w{p}", [NCHUNK, 128, 2 * E], mm_dts[p],
                          kind="ExternalInput") for p in range(n_pass)]
    pes_d = nc.dram_tensor("pes", [E], F32, kind="ExternalInput")
    probs_d = nc.dram_tensor("probs", [TPC, E], F32, kind="ExternalOutput")
    wi_d = nc.dram_tensor("wi", [TPC, 2 * TOPK], F32, kind="ExternalOutput")

    with tile.TileContext(nc) as tc:
        with (
            tc.tile_pool(name="const", bufs=1) as constp,
            tc.tile_pool(name="xin", bufs=B["xin"]) as xinp,
            tc.tile_pool(name="xt", bufs=B["xt"]) as xtp,
            tc.tile_pool(name="pst", bufs=B["pst"], space="PSUM") as pstp,
            tc.tile_pool(name="psl", bufs=B["psl"], space="PSUM") as pslp,
            tc.tile_pool(name="ssc", bufs=B["ssc"]) as sscp,
            tc.tile_pool(name="ex", bufs=B["ex"]) as exp_,
            tc.tile_pool(name="sm", bufs=B["sm"]) as smp,
            tc.tile_pool(name="outp", bufs=B["outp"]) as outp,
        ):
            # ---- constants ----
            ident = constp.tile([128, 128], F32)
            make_identity(nc, ident)
            iota_i = constp.tile([128, E], mybir.dt.int32)
            nc.gpsimd.iota(iota_i, pattern=[[1, E]], base=0,
                           channel_multiplier=0)
            iota_f = constp.tile([128, E], F32)
            nc.gpsimd.tensor_copy(iota_f, iota_i)
            pes_t = constp.tile([128, E], F32)
            nc.gpsimd.dma_start(
                out=pes_t,
                in_=bass.AP(tensor=pes_d, offset=0,
                            ap=[[0, 128], [1, E]]))
            sqb_t = constp.tile([128, 1], F32)
            nc.gpsimd.memset(sqb_t, sq_bias)
            w_sb = []
            for p in range(n_pass):
                wt = constp.tile([128, NCHUNK, 2 * E], mm_dts[p], tag=f"w{p}")
                for c in range(NCHUNK):
                    nc.sync.dma_start(out=wt[:, c, :], in_=w_d[p][c])
                w_sb.append(wt)

            state = {}

            def phase_a(m):
                """Heavy phase: DMA in, sumsq, transpose, matmuls, logits."""
                r0 = m * 128
                x_m = xinp.tile([128, HIDDEN], F32, tag="x_m")
                engs = [nc.sync, nc.scalar]
                n_eng = len(engs)
                step = HIDDEN // n_eng
                for j in range(n_eng):
                    e = engs[(m + j) % n_eng]
                    lo = j * step
                    hi = HIDDEN if j == n_eng - 1 else (j + 1) * step
                    e.dma_start(out=x_m[:, lo:hi], in_=x_d[r0:r0 + 128, lo:hi])

                if "ss" in ablate:
                    s_ap = smp.tile([128, 1], F32, tag="s_ap")
                    nc.vector.memset(s_ap, 1.0)
                else:
                    ss_a = smp.tile([128, 1], F32, tag="ss_a")
                    ss_v = smp.tile([128, 1], F32, tag="ss_v")
                    sc_a = sscp.tile([128, SS_ACT], F32, tag="sc_a")
                    sc_v = sscp.tile([128, HIDDEN - SS_ACT], F32, tag="sc_v")
                    nc.scalar.activation(sc_a, x_m[:, :SS_ACT], AOT.Square,
                                         accum_out=ss_a)
                    nc.vector.scalar_tensor_tensor(
                        out=sc_v, in0=x_m[:, SS_ACT:], scalar=1.0,
                        in1=x_m[:, SS_ACT:], op0=ALU.mult, op1=ALU.mult,
                        accum_out=ss_v)
                    nc.vector.tensor_add(ss_a, ss_a, ss_v)
                    sq_t = smp.tile([128, 1], F32, tag="sq_t")
                    nc.scalar.activation(sq_t, ss_a, AOT.Sqrt,
                                         scale=sq_scale, bias=sqb_t[:, :])
                    s_ap = smp.tile([128, 1], F32, tag="s_ap")
                    nc.vector.reciprocal(s_ap, sq_t)

                xt_m = xtp.tile([128, HIDDEN], xt_dt, tag="xt_m")
                pw = B["pst_w"]
                n_g = HIDDEN // pw
                for g in range(n_g if "transpose" not in ablate else 0):
                    ps_t = pstp.tile([128, pw], F32, tag="ps_t")
                    for cc in range(pw // 128):
                        c = g * (pw // 128) + cc
                        nc.tensor.transpose(
                            ps_t[:, cc * 128:(cc + 1) * 128],
                            x_m[:, c * 128:(c + 1) * 128], ident)
                    nc.scalar.copy(out=xt_m[:, g * pw:(g + 1) * pw],
                                   in_=ps_t)

                ps_l = pslp.tile([128, 2 * E], F32, tag="ps_l")
                if "mm" in ablate:
                    nc.vector.memset(ps_l, 0.01)
                for p in range(n_pass if "mm" not in ablate else 0):
                    for c in range(NCHUNK):
                        lhsT = xt_m[:, c * 128:(c + 1) * 128]
                        if scheme == "f32r":
                            lhsT = lhsT.bitcast(mybir.dt.float32r)
                        nc.tensor.matmul(
                            ps_l, lhsT, w_sb[p][:, c, :],
                            start=(p == 0 and c == 0),
                            stop=(p == n_pass - 1 and c == NCHUNK - 1))

                bvh_sb = exp_.tile([128, E], F32, tag="bvh_sb")
                nc.scalar.copy(out=bvh_sb, in_=ps_l[:, E:])
                state[m] = (ps_l, bvh_sb, s_ap)

            def phase_b(m):
                """Latency phase: top-32, top-2, softmax, weights, DMA out."""
                r0 = m * 128
                ps_l, bvh_sb, s_ap = state.pop(m)
                proj_l = ps_l[:, :E]
                bvh_l = bvh_sb

                scratch = exp_.tile([128, E], F32, tag="scratch")
                mx8 = smp.tile([128, 8], F32, tag="mx8")
                src = bvh_l
                if "top32" in ablate:
                    nc.vector.memset(scratch, NEG_BIG)
                for _ in range(NCAND // 8 if "top32" not in ablate else 0):
                    nc.vector.max(mx8, src)
                    nc.vector.match_replace(scratch, mx8, src, NEG_BIG)
                    src = scratch
                mask01 = exp_.tile([128, E], F32, tag="mask01")
                nc.gpsimd.tensor_scalar(
                    out=mask01, in0=scratch, scalar1=NEG_BIG, scalar2=None,
                    op0=ALU.is_equal)
                # t1 = +1e30 where candidate, -1e30 elsewhere
                t1 = exp_.tile([128, E], F32, tag="t1")
                nc.gpsimd.tensor_scalar(
                    out=t1, in0=mask01, scalar1=2e30, scalar2=-1e30,
                    op0=ALU.mult, op1=ALU.add)
                masked = exp_.tile([128, E], F32, tag="masked")
                nc.vector.tensor_tensor(
                    out=masked, in0=proj_l, in1=t1, op=ALU.min)

                top8 = smp.tile([128, 8], F32, tag="top8")
                idx8 = smp.tile([128, 8], mybir.dt.uint32, tag="idx8")
                nc.vector.max(top8, masked)
                nc.vector.max_index(idx8, top8, masked)

                probs = exp_.tile([128, E], F32, tag="probs")
                zsum = smp.tile([128, 1], F32, tag="zsum")
                nc.scalar.activation(probs, proj_l, AOT.Exp, scale=s_ap,
                                     accum_out=zsum)
                rz = smp.tile([128, 1], F32, tag="rz")
                nc.vector.reciprocal(rz, zsum)
                probs_o = outp.tile([128, E], F32, tag="probs_o")
                nc.gpsimd.tensor_scalar(
                    out=probs_o, in0=probs, scalar1=rz, scalar2=None,
                    op0=ALU.mult)
                nc.sync.dma_start(out=probs_d[r0:r0 + 128, :], in_=probs_o)

                v01 = smp.tile([128, 2], F32, tag="v01")
                nc.scalar.activation(v01, top8[:, 0:2], AOT.Exp, scale=s_ap)
                dsum = smp.tile([128, 1], F32, tag="dsum")
                nc.vector.tensor_add(dsum, v01[:, 0:1], v01[:, 1:2])
                rd = smp.tile([128, 1], F32, tag="rd")
                nc.vector.reciprocal(rd, dsum)
                idxf = smp.tile([128, 2], F32, tag="idxf")
                nc.gpsimd.tensor_copy(idxf, idx8[:, 0:2])
                wi = outp.tile([128, 4], F32, tag="wi")
                wv = wi[:, 0:2]
                junk = exp_.tile([128, E], F32, tag="junk")
                for k in range(TOPK):
                    gk = smp.tile([128, 1], F32, tag=f"gk{k}")
                    nc.vector.scalar_tensor_tensor(
                        out=junk, in0=iota_f, scalar=idxf[:, k:k + 1],
                        in1=pes_t, op0=ALU.is_equal, op1=ALU.mult,
                        accum_out=gk)
                    nc.vector.scalar_tensor_tensor(
                        out=wv[:, k:k + 1], in0=v01[:, k:k + 1],
                        scalar=rd, in1=gk, op0=ALU.mult, op1=ALU.mult)
                nc.gpsimd.tensor_copy(
                    wi[:, 2:4].bitcast(mybir.dt.uint32), idx8[:, 0:2])
                nc.sync.dma_start(out=wi_d[r0:r0 + 128, :], in_=wi)

            lag = B.get("lag", 2)
            loop_cm = (tc.For_i(0, repeat, 1) if repeat > 1
                       else contextlib.nullcontext())
            with loop_cm:
                for m in range(NSUB + lag):
                    if m < NSUB:
                        phase_a(m)
                    if m >= lag:
                        phase_b(m - lag)

    split_multi_waits(nc)
    return nc


def host_prep(norm_weight, scale, scalar_root_size, proj_w, bvh_w,
              scheme=SCHEME):
    """Fold norm weight + scalar calibration into the expert weights and
    lay them out as [chunk, 128, 2E] (proj | bvh concatenated on N)."""
    wpre = 256.0 if scheme == "fp16x2" else 1.0
    f = (norm_weight.astype(np.float64) * float(scale)
         * float(scalar_root_size) * wpre)
    wcat = np.concatenate([
        (proj_w.astype(np.float64) * f).T,      # [H, E]
        (bvh_w.astype(np.float64) * f).T,
    ], axis=1)                                   # [H, 2E]
    wcat32 = wcat.astype(np.float32)
    wcat32 = np.ascontiguousarray(
        wcat32.reshape(NCHUNK, 128, 2 * E))
    if scheme == "fp16x2":
        w1 = wcat32.astype(np.float16)
        w2 = (wcat32 - w1.astype(np.float32)).astype(np.float16)
        return [w1, w2]
    return [wcat32]


_prog_cache = {}


def _get_prog(scheme):
    if scheme not in _prog_cache:
        _prog_cache[scheme] = build_program(scheme)
    return _prog_cache[scheme]


def run(hidden_states, norm_weight, scale, scalar_root_size, proj_w, proj_b,
        bvh_w, bvh_b, per_expert_scale, scheme=SCHEME, trace=False,
        stitch_traces=False):
    assert not np.any(np.asarray(proj_b)) and not np.any(np.asarray(bvh_b)), \
        "nonzero router biases not supported by this kernel build"
    hs = np.ascontiguousarray(np.asarray(hidden_states, dtype=np.float32))
    w_list = host_prep(np.asarray(norm_weight), np.asarray(scale),
                       np.asarray(scalar_root_size), np.asarray(proj_w),
                       np.asarray(bvh_w), scheme)
    pes = np.ascontiguousarray(np.asarray(per_expert_scale, dtype=np.float32))

    nc = _get_prog(scheme)
    in_maps = []
    for c in range(N_CORES):
        m = {"x": hs[c * TPC:(c + 1) * TPC], "pes": pes}
        for p, w in enumerate(w_list):
            m[f"w{p}"] = w
        in_maps.append(m)
    res = run_bass_kernel_spmd(nc, in_maps, core_ids=list(range(N_CORES)),
                               trace=trace, stitch_traces=stitch_traces)
    probs = np.concatenate([r["probs"] for r in res.results], axis=0)
    wi = np.concatenate([r["wi"] for r in res.results], axis=0)
    wout = wi[:, 0:2]
    idx = wi[:, 2:4].view(np.uint32)
    return (probs, wout, idx.astype(np.int32)), res


def kernel(hidden_states, norm_weight, scale, scalar_root_size, proj_w,
           proj_b, bvh_w, bvh_b, per_expert_scale):
    out, _ = run(hidden_states, norm_weight, scale, scalar_root_size,
                 proj_w, proj_b, bvh_w, bvh_b, per_expert_scale)
    return out
